# revision 3
# baseline (speedup 1.0000x reference)
"""Trainium2 Bass kernel for windowed multi-head attention (nn_AttentionWindow).

Reference computation (B=64, N=197, DIM=768, H=12, HD=64):
    qkv  = x @ qkv_w.T + [q_bias, 0, v_bias]
    q, k, v = split(qkv);  q *= HD**-0.5
    attn = softmax(q @ k.T + rpb_table[rel_index])
    out  = (attn @ v) @ proj_w.T + proj_b

Sharding: data-parallel over batch across 8 NeuronCores (8 batches/core).

Per-core kernel design (all matmuls on TensorE, fp32 accumulation):
  - x is pre-transposed on host to xT [768, B*197] (feature-major).
  - QKV: q,k computed feature-major ([channels, tokens]) with fp32r
    matmuls over batch-pairs (N=394 >= 256 keeps fp32r at full rate);
    v computed token-major ([tokens, channels]) so the attention's
    P^T @ ... contraction has tokens on partitions; v stored bf16.
  - Scores are computed transposed: S^T[j,i] = k_h[:,j]^T q_h (fp32r,
    rhs padded to 256 cols). Softmax runs WITHOUT max subtraction
    (scores are O(1) by construction: q is pre-scaled by 1/8), as
    exp on ScalarE, then multiplied by the precomputed exp(bias)
    table (bf16) on VectorE:  P^T = exp(S^T) * exp(B^T).
  - O^T[d,i] = sum_j v[j,d] P^T[j,i] via bf16 matmuls; row sums are
    computed replicated across partitions with a ones-matrix matmul;
    reciprocal on ScalarE; normalization fused into the PSUM->SBUF
    copy on VectorE, assembling proj's rhs ([768, 394] per pair).
  - Projection: feature-major fp32r matmuls, output [768, tokens],
    transposed back on host.
"""
import sys
import functools

sys.path.insert(0, "/opt/trn_rl_repo")

import numpy as np
import ml_dtypes

import concourse.bass as bass  # noqa: E402
import concourse.bacc as bacc  # noqa: E402
import concourse.mybir as mybir  # noqa: E402
from concourse.tile import TileContext  # noqa: E402
from concourse.bass_utils import run_bass_kernel_spmd  # noqa: E402

F32 = mybir.dt.float32
F32R = mybir.dt.float32r
BF16 = mybir.dt.bfloat16

NCORES = 8
B, NT, DIM = 64, 197, 768
H, HD = 12, 64
SCALE = HD ** -0.5  # 0.125, exact power of two -> folded into q weights
BPC = B // NCORES   # 8 batches per core
SB = 4              # superbatches per core (one batch-pair each)
T2 = 2 * NT         # 394
KC = DIM // 128     # 6
QP = 256            # padded rhs width for fp32r score matmuls
QKW = 456           # qk sbuf tile width: batch1 pad reads up to 197+256=453


def build(qkv_bias_nonzero: bool, proj_bias_nonzero: bool):
    nc = bacc.Bacc("TRN2", target_bir_lowering=False, debug=False)

    xt = nc.dram_tensor("xt", [DIM, BPC * NT], F32R, kind="ExternalInput")
    qkw = nc.dram_tensor("qkw", [DIM, 2 * DIM], F32R, kind="ExternalInput")
    vw = nc.dram_tensor("vw", [DIM, DIM], F32R, kind="ExternalInput")
    pw = nc.dram_tensor("pw", [DIM, DIM], F32R, kind="ExternalInput")
    eb1 = nc.dram_tensor("eb1", [128, H * NT], BF16, kind="ExternalInput")
    eb2 = nc.dram_tensor("eb2", [NT - 128, H * NT], BF16, kind="ExternalInput")
    out = nc.dram_tensor("out", [DIM, BPC * NT], F32, kind="ExternalOutput")
    if qkv_bias_nonzero:
        qkb = nc.dram_tensor("qkb", [1, 2 * DIM], F32R, kind="ExternalInput")
        vb = nc.dram_tensor("vb", [1, DIM], F32R, kind="ExternalInput")
    if proj_bias_nonzero:
        pb = nc.dram_tensor("pb", [1, DIM], F32R, kind="ExternalInput")

    with TileContext(nc) as tc:
        with (
            tc.tile_pool(name="const", bufs=1) as constp,
            tc.tile_pool(name="xtp", bufs=9) as xtp,
            tc.tile_pool(name="qkp", bufs=18) as qkp,
            tc.tile_pool(name="vp", bufs=6) as vp,
            tc.tile_pool(name="pp", bufs=6) as pp,
            tc.tile_pool(name="rcp", bufs=3) as rcp,
            tc.tile_pool(name="opp", bufs=9) as opp,
            tc.tile_pool(name="obp", bufs=3) as obp,
            tc.tile_pool(name="ps", bufs=2, space="PSUM") as ps,
        ):
            # ---- resident constants ----
            qkw_s = constp.tile([128, KC, 2 * DIM], F32R, name="qkw_s")
            vw_s = constp.tile([128, KC, DIM], F32R, name="vw_s")
            pw_s = constp.tile([128, KC, DIM], F32R, name="pw_s")
            for kc in range(KC):
                nc.sync.dma_start(qkw_s[:, kc, :], qkw[kc * 128:(kc + 1) * 128, :])
                nc.sync.dma_start(vw_s[:, kc, :], vw[kc * 128:(kc + 1) * 128, :])
                nc.sync.dma_start(pw_s[:, kc, :], pw[kc * 128:(kc + 1) * 128, :])
            eb1_s = constp.tile([128, H * NT], BF16, name="eb1_s")
            eb2_s = constp.tile([NT - 128, H * NT], BF16, name="eb2_s")
            nc.sync.dma_start(eb1_s[:, :], eb1[:, :])
            nc.sync.dma_start(eb2_s[:, :], eb2[:, :])
            ones_bf = constp.tile([128, 128], BF16, name="ones_bf")
            nc.gpsimd.memset(ones_bf[:, :], 1.0)
            if qkv_bias_nonzero:
                qkb_s = constp.tile([1, 2 * DIM], F32R, name="qkb_s")
                vb_s = constp.tile([1, DIM], F32R, name="vb_s")
                nc.sync.dma_start(qkb_s[:, :], qkb[:, :])
                nc.sync.dma_start(vb_s[:, :], vb[:, :])
            if proj_bias_nonzero:
                pb_s = constp.tile([1, DIM], F32R, name="pb_s")
                nc.sync.dma_start(pb_s[:, :], pb[:, :])
            if qkv_bias_nonzero or proj_bias_nonzero:
                ones_f = constp.tile([1, T2], F32R, name="ones_f")
                nc.gpsimd.memset(ones_f[:, :], 1.0)

            for sb in range(SB):
                # ---- load xT chunk for this batch pair ----
                xts = []
                for kc in range(KC):
                    xtt = xtp.tile([128, T2], F32R, name="xtt", tag="xt")
                    nc.sync.dma_start(
                        xtt[:, :],
                        xt[kc * 128:(kc + 1) * 128, sb * T2:(sb + 1) * T2])
                    xts.append(xtt)

                # ---- q,k feature-major: [ch 1536 -> 12 chunks, 394 tok] ----
                qkt = []
                for c in range(2 * KC):
                    acc = ps.tile([128, T2], F32, name="acc_qk", tag="mm")
                    for kc in range(KC):
                        nc.tensor.matmul(
                            acc[:, :],
                            qkw_s[:, kc, c * 128:(c + 1) * 128],
                            xts[kc][:, :],
                            start=(kc == 0),
                            stop=(kc == KC - 1) and not qkv_bias_nonzero,
                        )
                    if qkv_bias_nonzero:
                        nc.tensor.matmul(
                            acc[:, :],
                            qkb_s[0:1, c * 128:(c + 1) * 128],
                            ones_f[0:1, :],
                            start=False, stop=True,
                        )
                    t = qkp.tile([128, QKW], F32R, name="qk_t", tag="qk")
                    nc.scalar.copy(t[:, 0:T2], acc[:, :])
                    qkt.append(t)

                # ---- v token-major: [394 tok -> 2x(128,69) rows, 768 ch] ----
                vt = [[None, None], [None, None]]
                for b01 in range(2):
                    for tch in range(2):
                        toff = b01 * NT + tch * 128
                        tlen = 128 if tch == 0 else NT - 128
                        acc = ps.tile([128, DIM], F32, name="acc_v", tag="mm")
                        for half in range(2):
                            n0, n1 = half * 512, min(DIM, (half + 1) * 512)
                            for kc in range(KC):
                                nc.tensor.matmul(
                                    acc[0:tlen, n0:n1],
                                    xts[kc][:, toff:toff + tlen],
                                    vw_s[:, kc, n0:n1],
                                    start=(kc == 0),
                                    stop=(kc == KC - 1) and not qkv_bias_nonzero,
                                )
                            if qkv_bias_nonzero:
                                nc.tensor.matmul(
                                    acc[0:tlen, n0:n1],
                                    ones_f[0:1, 0:tlen],
                                    vb_s[0:1, n0:n1],
                                    start=False, stop=True,
                                )
                        t = vp.tile([128, DIM], BF16, name="v_t", tag="v")
                        nc.vector.tensor_copy(t[0:tlen, :], acc[0:tlen, :])
                        vt[b01][tch] = t

                # ---- O' assembly tiles: proj rhs [768 ch -> 6 chunks, 394] ----
                oprime = [opp.tile([128, T2], F32R, name="opr", tag="opr")
                          for _ in range(KC)]

                # ---- attention per (batch, head) ----
                for b01 in range(2):
                    for h in range(H):
                        qc, row0 = h // 2, (h % 2) * 64
                        q_ap = qkt[qc][row0:row0 + 64,
                                       b01 * NT:b01 * NT + QP]
                        kt = qkt[KC + qc]
                        s1 = ps.tile([128, QP], F32, name="s1", tag="attn")
                        nc.tensor.matmul(
                            s1[:, :],
                            kt[row0:row0 + 64, b01 * NT:b01 * NT + 128],
                            q_ap, start=True, stop=True)
                        s2 = ps.tile([128, QP], F32, name="s2", tag="attn")
                        nc.tensor.matmul(
                            s2[0:NT - 128, :],
                            kt[row0:row0 + 64,
                               b01 * NT + 128:b01 * NT + NT],
                            q_ap, start=True, stop=True)

                        p1 = pp.tile([128, NT], BF16, name="p1", tag="p")
                        nc.scalar.activation(
                            p1[:, :], s1[:, 0:NT],
                            mybir.ActivationFunctionType.Exp)
                        p2 = pp.tile([128, NT], BF16, name="p2", tag="p")
                        nc.scalar.activation(
                            p2[0:NT - 128, :], s2[0:NT - 128, 0:NT],
                            mybir.ActivationFunctionType.Exp)
                        nc.vector.tensor_mul(
                            p1[:, :], p1[:, :],
                            eb1_s[:, h * NT:(h + 1) * NT])
                        nc.vector.tensor_mul(
                            p2[0:NT - 128, :], p2[0:NT - 128, :],
                            eb2_s[:, h * NT:(h + 1) * NT])

                        tp = (0, row0) if row0 else None
                        oacc = ps.tile([128, QP], F32, name="oacc", tag="attn")
                        nc.tensor.matmul(
                            oacc[row0:row0 + 64, 0:NT],
                            vt[b01][0][:, h * HD:(h + 1) * HD],
                            p1[:, :], start=True, stop=False,
                            tile_position=tp)
                        nc.tensor.matmul(
                            oacc[row0:row0 + 64, 0:NT],
                            vt[b01][1][0:NT - 128, h * HD:(h + 1) * HD],
                            p2[0:NT - 128, :], start=False, stop=True,
                            tile_position=tp)

                        sacc = ps.tile([128, QP], F32, name="sacc", tag="attn")
                        nc.tensor.matmul(
                            sacc[:, 0:NT], ones_bf[:, :], p1[:, :],
                            start=True, stop=False)
                        nc.tensor.matmul(
                            sacc[:, 0:NT], ones_bf[0:NT - 128, :],
                            p2[0:NT - 128, :], start=False, stop=True)

                        rct = rcp.tile([128, NT], F32, name="rct", tag="rc")
                        nc.vector.reciprocal_approx_fast(
                            out=rct[:, :], in_=sacc[:, 0:NT])
                        nc.vector.tensor_mul(
                            oprime[qc][row0:row0 + 64, b01 * NT:(b01 + 1) * NT],
                            oacc[row0:row0 + 64, 0:NT],
                            rct[row0:row0 + 64, :])

                # ---- projection: out_fm [768 -> 6 chunks, 394] ----
                for c in range(KC):
                    acc = ps.tile([128, T2], F32, name="acc_p", tag="mm")
                    for kp in range(KC):
                        nc.tensor.matmul(
                            acc[:, :],
                            pw_s[:, kp, c * 128:(c + 1) * 128],
                            oprime[kp][:, :],
                            start=(kp == 0),
                            stop=(kp == KC - 1) and not proj_bias_nonzero,
                        )
                    if proj_bias_nonzero:
                        nc.tensor.matmul(
                            acc[:, :],
                            pb_s[0:1, c * 128:(c + 1) * 128],
                            ones_f[0:1, :],
                            start=False, stop=True,
                        )
                    obt = obp.tile([128, T2], F32, name="obt", tag="ob")
                    nc.scalar.copy(obt[:, :], acc[:, :])
                    nc.sync.dma_start(
                        out[c * 128:(c + 1) * 128, sb * T2:(sb + 1) * T2],
                        obt[:, :])

    nc.compile()
    return nc


@functools.lru_cache(maxsize=4)
def _built(qkv_bias_nonzero: bool, proj_bias_nonzero: bool):
    return build(qkv_bias_nonzero, proj_bias_nonzero)


def prepare_inputs(x, qkv_w, q_bias, v_bias, rpb_table, proj_w, proj_b, rel_index):
    """Host-side prep: shard + transpose + fold scale + gather bias table."""
    x = np.asarray(x, dtype=np.float32)
    qkv_w = np.asarray(qkv_w, dtype=np.float32)
    q_bias = np.asarray(q_bias, dtype=np.float32)
    v_bias = np.asarray(v_bias, dtype=np.float32)
    rpb_table = np.asarray(rpb_table, dtype=np.float32)
    proj_w = np.asarray(proj_w, dtype=np.float32)
    proj_b = np.asarray(proj_b, dtype=np.float32)
    rel_index = np.asarray(rel_index)

    qw = qkv_w[0:DIM] * np.float32(SCALE)   # exact: SCALE is a power of two
    qkw_h = np.ascontiguousarray(
        np.concatenate([qw, qkv_w[DIM:2 * DIM]], axis=0).T)      # [768, 1536]
    vw_h = np.ascontiguousarray(qkv_w[2 * DIM:3 * DIM].T)        # [768, 768]
    pw_h = np.ascontiguousarray(proj_w.T)                        # [768, 768]

    # bias[i, j, h] -> exp -> ebT[h, j, i]
    bias = rpb_table[rel_index]                                  # (197,197,12)
    ebT = np.exp(bias.astype(np.float32)).transpose(2, 1, 0)     # (12, j, i)
    eb1_h = np.ascontiguousarray(
        ebT[:, 0:128, :].transpose(1, 0, 2).reshape(128, H * NT)
    ).astype(ml_dtypes.bfloat16)
    eb2_h = np.ascontiguousarray(
        ebT[:, 128:NT, :].transpose(1, 0, 2).reshape(NT - 128, H * NT)
    ).astype(ml_dtypes.bfloat16)

    qkv_bias_nonzero = bool(q_bias.any() or v_bias.any())
    proj_bias_nonzero = bool(proj_b.any())

    in_maps = []
    for i in range(NCORES):
        xs = x[i * BPC:(i + 1) * BPC].reshape(BPC * NT, DIM)
        m = {
            "xt": np.ascontiguousarray(xs.T),
            "qkw": qkw_h, "vw": vw_h, "pw": pw_h,
            "eb1": eb1_h, "eb2": eb2_h,
        }
        if qkv_bias_nonzero:
            m["qkb"] = np.concatenate(
                [q_bias * np.float32(SCALE), np.zeros_like(q_bias)])[None, :]
            # careful: kernel's qk bias layout is [scaled q_bias | k_bias(=0)]
            m["qkb"] = np.ascontiguousarray(m["qkb"], dtype=np.float32)
            m["vb"] = np.ascontiguousarray(v_bias[None, :], dtype=np.float32)
        if proj_bias_nonzero:
            m["pb"] = np.ascontiguousarray(proj_b[None, :], dtype=np.float32)
        in_maps.append(m)
    return in_maps, qkv_bias_nonzero, proj_bias_nonzero


def kernel(x, qkv_w, q_bias, v_bias, rpb_table, proj_w, proj_b, rel_index):
    in_maps, qb_nz, pb_nz = prepare_inputs(
        x, qkv_w, q_bias, v_bias, rpb_table, proj_w, proj_b, rel_index)
    nc = _built(qb_nz, pb_nz)
    res = run_bass_kernel_spmd(nc, in_maps, core_ids=list(range(NCORES)))
    outs = []
    for i in range(NCORES):
        ofm = res.results[i]["out"]                  # [768, 1576]
        outs.append(ofm.T.reshape(BPC, NT, DIM))
    return np.concatenate(outs, axis=0).astype(np.float32)


# revision 4
# speedup vs baseline: 1.1523x; 1.1523x over previous
"""Trainium2 Bass kernel for windowed multi-head attention (nn_AttentionWindow).

Reference computation (B=64, N=197, DIM=768, H=12, HD=64):
    qkv  = x @ qkv_w.T + [q_bias, 0, v_bias]
    q, k, v = split(qkv);  q *= HD**-0.5
    attn = softmax(q @ k.T + rpb_table[rel_index])
    out  = (attn @ v) @ proj_w.T + proj_b

Sharding: data-parallel over batch across 8 NeuronCores (8 batches/core).

Per-core design (all matmuls on TensorE, fp32 PSUM accumulation):
  - x pre-transposed on host to xT [768, 1576] (feature-major); work
    split into 4 superbatches of one batch-pair (matmul N = 394).
  - q,k: feature-major fp32r matmuls (weights pre-scaled by 1/8 for q),
    PSUM copied to bf16 SBUF tiles (attention runs bf16).
  - v: token-major bf16 matmuls ([tokens, channels]) so the attention
    contraction has tokens on partitions; stored bf16.
  - Scores transposed: S^T[j,i] = k_h[:,j]^T q_h, bf16, N=197. Softmax
    WITHOUT max subtraction (scores are O(1): q pre-scaled), exp on
    ScalarE -> P bf16, multiplied by precomputed exp(bias) (bf16) on
    VectorE: softmax(S+B) = expS*expB / sum(expS*expB).
  - O^T[d,i] = sum_j v[j,d] P^T[j,i] (bf16); row sums replicated
    across partitions via ones-matrix matmul into the same PSUM bank;
    reciprocal_approx_fast on VectorE; normalization fused into the
    PSUM->SBUF copy assembling proj's rhs ([768, 394] per pair, f32r).
  - Attention is software-pipelined (skew over (head, batch) items,
    single-bank PSUM tiles) to keep TensorE dense and HAM-warm.
  - Projection: feature-major fp32r matmuls; host transposes back.
"""
import sys
import functools

sys.path.insert(0, "/opt/trn_rl_repo")

import numpy as np
import ml_dtypes

import concourse.bass as bass  # noqa: E402
import concourse.bacc as bacc  # noqa: E402
import concourse.mybir as mybir  # noqa: E402
from concourse.tile import TileContext  # noqa: E402
from concourse.bass_utils import run_bass_kernel_spmd  # noqa: E402

F32 = mybir.dt.float32
F32R = mybir.dt.float32r
BF16 = mybir.dt.bfloat16

NCORES = 8
B, NT, DIM = 64, 197, 768
H, HD = 12, 64
SCALE = HD ** -0.5  # 0.125, exact power of two -> folded into q weights
BPC = B // NCORES   # 8 batches per core
SB = 4              # superbatches per core (one batch-pair each)
T2 = 2 * NT         # 394
KC = DIM // 128     # 6
NT2 = NT - 128      # 69 (second token chunk)
SKEW = 3            # attention software-pipeline depth


def build(qkv_bias_nonzero: bool, proj_bias_nonzero: bool):
    nc = bacc.Bacc("TRN2", target_bir_lowering=False, debug=False)

    xt = nc.dram_tensor("xt", [DIM, BPC * NT], F32R, kind="ExternalInput")
    qkw = nc.dram_tensor("qkw", [DIM, 2 * DIM], F32R, kind="ExternalInput")
    vw = nc.dram_tensor("vw", [DIM, DIM], BF16, kind="ExternalInput")
    pw = nc.dram_tensor("pw", [DIM, DIM], F32R, kind="ExternalInput")
    eb1 = nc.dram_tensor("eb1", [128, H * NT], BF16, kind="ExternalInput")
    eb2 = nc.dram_tensor("eb2", [NT2, H * NT], BF16, kind="ExternalInput")
    out = nc.dram_tensor("out", [DIM, BPC * NT], F32, kind="ExternalOutput")
    if qkv_bias_nonzero:
        qkb = nc.dram_tensor("qkb", [1, 2 * DIM], F32R, kind="ExternalInput")
        vb = nc.dram_tensor("vb", [1, DIM], BF16, kind="ExternalInput")
    if proj_bias_nonzero:
        pb = nc.dram_tensor("pb", [1, DIM], F32R, kind="ExternalInput")

    with TileContext(nc) as tc:
        with (
            tc.tile_pool(name="const", bufs=1) as constp,
            tc.tile_pool(name="xtp", bufs=9) as xtp,
            tc.tile_pool(name="xbp", bufs=9) as xbp,
            tc.tile_pool(name="qkp", bufs=18) as qkp,
            tc.tile_pool(name="vp", bufs=6) as vp,
            tc.tile_pool(name="pp", bufs=2 * (SKEW + 2)) as pp,
            tc.tile_pool(name="rcp", bufs=3) as rcp,
            tc.tile_pool(name="opp", bufs=9) as opp,
            tc.tile_pool(name="obp", bufs=3) as obp,
            tc.tile_pool(name="ps", bufs=1, space="PSUM") as ps,
            tc.tile_pool(name="psa", bufs=6, space="PSUM") as psa,
        ):
            # ---- resident constants ----
            qkw_s = constp.tile([128, KC, 2 * DIM], F32R, name="qkw_s")
            vw_s = constp.tile([128, KC, DIM], BF16, name="vw_s")
            pw_s = constp.tile([128, KC, DIM], F32R, name="pw_s")
            for kc in range(KC):
                nc.sync.dma_start(qkw_s[:, kc, :], qkw[kc * 128:(kc + 1) * 128, :])
                nc.sync.dma_start(vw_s[:, kc, :], vw[kc * 128:(kc + 1) * 128, :])
                nc.sync.dma_start(pw_s[:, kc, :], pw[kc * 128:(kc + 1) * 128, :])
            eb1_s = constp.tile([128, H * NT], BF16, name="eb1_s")
            eb2_s = constp.tile([NT2, H * NT], BF16, name="eb2_s")
            nc.sync.dma_start(eb1_s[:, :], eb1[:, :])
            nc.sync.dma_start(eb2_s[:, :], eb2[:, :])
            ones_bf = constp.tile([128, 128], BF16, name="ones_bf")
            nc.gpsimd.memset(ones_bf[:, :], 1.0)
            if qkv_bias_nonzero:
                qkb_s = constp.tile([1, 2 * DIM], F32R, name="qkb_s")
                vb_s = constp.tile([1, DIM], BF16, name="vb_s")
                nc.sync.dma_start(qkb_s[:, :], qkb[:, :])
                nc.sync.dma_start(vb_s[:, :], vb[:, :])
            if proj_bias_nonzero:
                pb_s = constp.tile([1, DIM], F32R, name="pb_s")
                nc.sync.dma_start(pb_s[:, :], pb[:, :])
            if qkv_bias_nonzero or proj_bias_nonzero:
                ones_fr = constp.tile([1, T2], F32R, name="ones_fr")
                nc.gpsimd.memset(ones_fr[:, :], 1.0)
                ones_bfr = constp.tile([1, T2], BF16, name="ones_bfr")
                nc.gpsimd.memset(ones_bfr[:, :], 1.0)

            for sb in range(SB):
                # ---- load xT chunks for this batch pair ----
                xts = []
                xbs = []
                for kc in range(KC):
                    xtt = xtp.tile([128, T2], F32R, name="xtt", tag="xt")
                    nc.sync.dma_start(
                        xtt[:, :],
                        xt[kc * 128:(kc + 1) * 128, sb * T2:(sb + 1) * T2])
                    xts.append(xtt)
                    xbt = xbp.tile([128, T2], BF16, name="xbt", tag="xb")
                    nc.vector.tensor_copy(xbt[:, :], xtt[:, :].bitcast(F32))
                    xbs.append(xbt)

                # ---- q,k feature-major fp32r: [1536ch -> 12 chunks, 394] ----
                qkt = []
                for c in range(2 * KC):
                    acc = ps.tile([128, T2], F32, name="acc_qk", tag="mm")
                    for kc in range(KC):
                        nc.tensor.matmul(
                            acc[:, :],
                            qkw_s[:, kc, c * 128:(c + 1) * 128],
                            xts[kc][:, :],
                            start=(kc == 0),
                            stop=(kc == KC - 1) and not qkv_bias_nonzero,
                        )
                    if qkv_bias_nonzero:
                        nc.tensor.matmul(
                            acc[:, :],
                            qkb_s[0:1, c * 128:(c + 1) * 128],
                            ones_fr[0:1, :],
                            start=False, stop=True,
                        )
                    t = qkp.tile([128, T2], BF16, name="qk_t", tag="qk")
                    nc.scalar.copy(t[:, :], acc[:, :])
                    qkt.append(t)

                # ---- v token-major bf16: [394 tok -> 2x(128,69), 768ch] ----
                vt = [[None, None], [None, None]]
                for b01 in range(2):
                    for tch in range(2):
                        toff = b01 * NT + tch * 128
                        tlen = 128 if tch == 0 else NT2
                        acc = ps.tile([128, DIM], F32, name="acc_v", tag="mm")
                        for half in range(2):
                            n0, n1 = half * 512, min(DIM, (half + 1) * 512)
                            for kc in range(KC):
                                nc.tensor.matmul(
                                    acc[0:tlen, n0:n1],
                                    xbs[kc][:, toff:toff + tlen],
                                    vw_s[:, kc, n0:n1],
                                    start=(kc == 0),
                                    stop=(kc == KC - 1) and not qkv_bias_nonzero,
                                )
                            if qkv_bias_nonzero:
                                nc.tensor.matmul(
                                    acc[0:tlen, n0:n1],
                                    ones_bfr[0:1, 0:tlen],
                                    vb_s[0:1, n0:n1],
                                    start=False, stop=True,
                                )
                        t = vp.tile([128, DIM], BF16, name="v_t", tag="v")
                        nc.vector.tensor_copy(t[0:tlen, :], acc[0:tlen, :])
                        vt[b01][tch] = t

                # ---- O' assembly tiles: proj rhs [768ch -> 6 chunks, 394] ----
                oprime = [opp.tile([128, T2], F32R, name="opr", tag="opr")
                          for _ in range(KC)]

                # ---- attention, software-pipelined over (h, b01) ----
                def stage_a(b01, h):
                    """S^T matmuls + exp + bias-mult -> P^T (bf16)."""
                    qc, row0 = h // 2, (h % 2) * 64
                    q_ap = qkt[qc][row0:row0 + 64, b01 * NT:(b01 + 1) * NT]
                    kt = qkt[KC + qc]
                    st = psa.tile([128, 512], F32, name="st", tag="attn")
                    nc.tensor.matmul(
                        st[:, 0:NT],
                        kt[row0:row0 + 64, b01 * NT:b01 * NT + 128],
                        q_ap, start=True, stop=True)
                    nc.tensor.matmul(
                        st[0:NT2, 256:256 + NT],
                        kt[row0:row0 + 64, b01 * NT + 128:(b01 + 1) * NT],
                        q_ap, start=True, stop=True)
                    p1 = pp.tile([128, NT], BF16, name="p1", tag="p")
                    nc.scalar.activation(
                        p1[:, :], st[:, 0:NT],
                        mybir.ActivationFunctionType.Exp)
                    p2 = pp.tile([128, NT], BF16, name="p2", tag="p")
                    nc.scalar.activation(
                        p2[0:NT2, :], st[0:NT2, 256:256 + NT],
                        mybir.ActivationFunctionType.Exp)
                    nc.vector.tensor_mul(
                        p1[:, :], p1[:, :], eb1_s[:, h * NT:(h + 1) * NT])
                    nc.vector.tensor_mul(
                        p2[0:NT2, :], p2[0:NT2, :],
                        eb2_s[:, h * NT:(h + 1) * NT])
                    return p1, p2

                def stage_b(b01, h, p1, p2):
                    """O^T + replicated sums + reciprocal + normalize."""
                    qc, row0 = h // 2, (h % 2) * 64
                    tp = (0, row0) if row0 else None
                    ot = psa.tile([128, 512], F32, name="ot", tag="attn")
                    nc.tensor.matmul(
                        ot[row0:row0 + 64, 0:NT],
                        vt[b01][0][:, h * HD:(h + 1) * HD],
                        p1[:, :], start=True, stop=False,
                        tile_position=tp)
                    nc.tensor.matmul(
                        ot[row0:row0 + 64, 0:NT],
                        vt[b01][1][0:NT2, h * HD:(h + 1) * HD],
                        p2[0:NT2, :], start=False, stop=True,
                        tile_position=tp)
                    nc.tensor.matmul(
                        ot[:, 256:256 + NT], ones_bf[:, :], p1[:, :],
                        start=True, stop=False)
                    nc.tensor.matmul(
                        ot[:, 256:256 + NT], ones_bf[0:NT2, :],
                        p2[0:NT2, :], start=False, stop=True)
                    rct = rcp.tile([128, NT], F32, name="rct", tag="rc")
                    nc.vector.reciprocal_approx_fast(
                        out=rct[:, :], in_=ot[:, 256:256 + NT])
                    nc.vector.tensor_mul(
                        oprime[qc][row0:row0 + 64, b01 * NT:(b01 + 1) * NT],
                        ot[row0:row0 + 64, 0:NT],
                        rct[row0:row0 + 64, :])

                items = [(i // 2, i % 2) for i in range(2 * H)]  # (h, b01)
                pend = {}
                for i, (h, b01) in enumerate(items):
                    pend[i] = (b01, h) + stage_a(b01, h)
                    if i >= SKEW:
                        stage_b(*pend.pop(i - SKEW))
                for i in sorted(pend):
                    stage_b(*pend.pop(i))

                # ---- projection fp32r: out_fm [768 -> 6 chunks, 394] ----
                for c in range(KC):
                    acc = ps.tile([128, T2], F32, name="acc_p", tag="mm")
                    for kp in range(KC):
                        nc.tensor.matmul(
                            acc[:, :],
                            pw_s[:, kp, c * 128:(c + 1) * 128],
                            oprime[kp][:, :],
                            start=(kp == 0),
                            stop=(kp == KC - 1) and not proj_bias_nonzero,
                        )
                    if proj_bias_nonzero:
                        nc.tensor.matmul(
                            acc[:, :],
                            pb_s[0:1, c * 128:(c + 1) * 128],
                            ones_fr[0:1, :],
                            start=False, stop=True,
                        )
                    obt = obp.tile([128, T2], F32, name="obt", tag="ob")
                    nc.scalar.copy(obt[:, :], acc[:, :])
                    nc.sync.dma_start(
                        out[c * 128:(c + 1) * 128, sb * T2:(sb + 1) * T2],
                        obt[:, :])

    nc.compile()
    return nc


@functools.lru_cache(maxsize=4)
def _built(qkv_bias_nonzero: bool, proj_bias_nonzero: bool):
    return build(qkv_bias_nonzero, proj_bias_nonzero)


def prepare_inputs(x, qkv_w, q_bias, v_bias, rpb_table, proj_w, proj_b, rel_index):
    """Host-side prep: shard + transpose + fold scale + gather bias table."""
    x = np.asarray(x, dtype=np.float32)
    qkv_w = np.asarray(qkv_w, dtype=np.float32)
    q_bias = np.asarray(q_bias, dtype=np.float32)
    v_bias = np.asarray(v_bias, dtype=np.float32)
    rpb_table = np.asarray(rpb_table, dtype=np.float32)
    proj_w = np.asarray(proj_w, dtype=np.float32)
    proj_b = np.asarray(proj_b, dtype=np.float32)
    rel_index = np.asarray(rel_index)

    qw = qkv_w[0:DIM] * np.float32(SCALE)   # exact: SCALE is a power of two
    qkw_h = np.ascontiguousarray(
        np.concatenate([qw, qkv_w[DIM:2 * DIM]], axis=0).T)      # [768, 1536]
    vw_h = np.ascontiguousarray(qkv_w[2 * DIM:3 * DIM].T).astype(
        ml_dtypes.bfloat16)                                      # [768, 768]
    pw_h = np.ascontiguousarray(proj_w.T)                        # [768, 768]

    # bias[i, j, h] -> exp -> ebT[h, j, i]
    bias = rpb_table[rel_index]                                  # (197,197,12)
    ebT = np.exp(bias.astype(np.float32)).transpose(2, 1, 0)     # (12, j, i)
    eb1_h = np.ascontiguousarray(
        ebT[:, 0:128, :].transpose(1, 0, 2).reshape(128, H * NT)
    ).astype(ml_dtypes.bfloat16)
    eb2_h = np.ascontiguousarray(
        ebT[:, 128:NT, :].transpose(1, 0, 2).reshape(NT2, H * NT)
    ).astype(ml_dtypes.bfloat16)

    qkv_bias_nonzero = bool(q_bias.any() or v_bias.any())
    proj_bias_nonzero = bool(proj_b.any())

    in_maps = []
    for i in range(NCORES):
        xs = x[i * BPC:(i + 1) * BPC].reshape(BPC * NT, DIM)
        m = {
            "xt": np.ascontiguousarray(xs.T),
            "qkw": qkw_h, "vw": vw_h, "pw": pw_h,
            "eb1": eb1_h, "eb2": eb2_h,
        }
        if qkv_bias_nonzero:
            m["qkb"] = np.ascontiguousarray(
                np.concatenate([q_bias * np.float32(SCALE),
                                np.zeros_like(q_bias)])[None, :],
                dtype=np.float32)
            m["vb"] = np.ascontiguousarray(
                v_bias[None, :]).astype(ml_dtypes.bfloat16)
        if proj_bias_nonzero:
            m["pb"] = np.ascontiguousarray(proj_b[None, :], dtype=np.float32)
        in_maps.append(m)
    return in_maps, qkv_bias_nonzero, proj_bias_nonzero


def kernel(x, qkv_w, q_bias, v_bias, rpb_table, proj_w, proj_b, rel_index):
    in_maps, qb_nz, pb_nz = prepare_inputs(
        x, qkv_w, q_bias, v_bias, rpb_table, proj_w, proj_b, rel_index)
    nc = _built(qb_nz, pb_nz)
    res = run_bass_kernel_spmd(nc, in_maps, core_ids=list(range(NCORES)))
    outs = []
    for i in range(NCORES):
        ofm = res.results[i]["out"]                  # [768, 1576]
        outs.append(ofm.T.reshape(BPC, NT, DIM))
    return np.concatenate(outs, axis=0).astype(np.float32)


# revision 5
# speedup vs baseline: 1.4449x; 1.2540x over previous
"""Trainium2 Bass kernel for windowed multi-head attention (nn_AttentionWindow).

Reference computation (B=64, N=197, DIM=768, H=12, HD=64):
    qkv  = x @ qkv_w.T + [q_bias, 0, v_bias]
    q, k, v = split(qkv);  q *= HD**-0.5
    attn = softmax(q @ k.T + rpb_table[rel_index])
    out  = (attn @ v) @ proj_w.T + proj_b

Sharding: data-parallel over batch across 8 NeuronCores (8 batches/core).

Per-core design (all matmuls on TensorE, fp32 PSUM accumulation):
  - x pre-transposed on host to xT [768, 1576] (feature-major); work
    split into 4 superbatches of one batch-pair (matmul N = 394).
  - q,k: feature-major fp32r matmuls (weights pre-scaled by 1/8 for q),
    PSUM copied to bf16 SBUF tiles (attention runs bf16).
  - v: token-major bf16 matmuls ([tokens, channels]) so the attention
    contraction has tokens on partitions; stored bf16.
  - Scores transposed: S^T[j,i] = k_h[:,j]^T q_h, bf16, N=197. Softmax
    WITHOUT max subtraction (scores are O(1): q pre-scaled), exp on
    ScalarE -> P bf16, multiplied by precomputed exp(bias) (bf16) on
    VectorE: softmax(S+B) = expS*expB / sum(expS*expB).
  - O^T[d,i] = sum_j v[j,d] P^T[j,i] (bf16); row sums replicated
    across partitions via ones-matrix matmul into the same PSUM bank;
    reciprocal_approx_fast on VectorE; normalization fused into the
    PSUM->SBUF copy assembling proj's rhs ([768, 394] per pair, f32r).
  - Attention is software-pipelined (skew over (head, batch) items,
    single-bank PSUM tiles) to keep TensorE dense and HAM-warm.
  - Projection: feature-major fp32r matmuls; host transposes back.
"""
import sys
import functools

sys.path.insert(0, "/opt/trn_rl_repo")

import numpy as np
import ml_dtypes

import concourse.bass as bass  # noqa: E402
import concourse.bacc as bacc  # noqa: E402
import concourse.mybir as mybir  # noqa: E402
from concourse.tile import TileContext  # noqa: E402
from concourse.bass_utils import run_bass_kernel_spmd  # noqa: E402

F32 = mybir.dt.float32
F32R = mybir.dt.float32r
BF16 = mybir.dt.bfloat16

NCORES = 8
B, NT, DIM = 64, 197, 768
H, HD = 12, 64
SCALE = HD ** -0.5  # 0.125, exact power of two -> folded into q weights
BPC = B // NCORES   # 8 batches per core
SB = 4              # superbatches per core (one batch-pair each)
T2 = 2 * NT         # 394
KC = DIM // 128     # 6
NT2 = NT - 128      # 69 (second token chunk)
SKEW = 3            # attention software-pipeline depth


def build(qkv_bias_nonzero: bool, proj_bias_nonzero: bool):
    nc = bacc.Bacc("TRN2", target_bir_lowering=False, debug=False)

    xt = nc.dram_tensor("xt", [DIM, BPC * NT], BF16, kind="ExternalInput")
    qkw = nc.dram_tensor("qkw", [DIM, 2 * DIM], BF16, kind="ExternalInput")
    vw = nc.dram_tensor("vw", [DIM, DIM], BF16, kind="ExternalInput")
    pw = nc.dram_tensor("pw", [DIM, DIM], BF16, kind="ExternalInput")
    eb1 = nc.dram_tensor("eb1", [128, H * NT], BF16, kind="ExternalInput")
    eb2 = nc.dram_tensor("eb2", [NT2, H * NT], BF16, kind="ExternalInput")
    out = nc.dram_tensor("out", [DIM, BPC * NT], F32, kind="ExternalOutput")
    if qkv_bias_nonzero:
        qkb = nc.dram_tensor("qkb", [1, 2 * DIM], BF16, kind="ExternalInput")
        vb = nc.dram_tensor("vb", [1, DIM], BF16, kind="ExternalInput")
    if proj_bias_nonzero:
        pb = nc.dram_tensor("pb", [1, DIM], BF16, kind="ExternalInput")

    with TileContext(nc) as tc:
        with (
            tc.tile_pool(name="const", bufs=1) as constp,
            tc.tile_pool(name="xbp", bufs=9) as xbp,
            tc.tile_pool(name="qkp", bufs=18) as qkp,
            tc.tile_pool(name="vp", bufs=6) as vp,
            tc.tile_pool(name="pp", bufs=2 * (SKEW + 2)) as pp,
            tc.tile_pool(name="rcp", bufs=3) as rcp,
            tc.tile_pool(name="opp", bufs=9) as opp,
            tc.tile_pool(name="obp", bufs=3) as obp,
            tc.tile_pool(name="ps", bufs=1, space="PSUM") as ps,
            tc.tile_pool(name="psa", bufs=6, space="PSUM") as psa,
        ):
            # ---- resident constants ----
            qkw_s = constp.tile([128, KC, 2 * DIM], BF16, name="qkw_s")
            vw_s = constp.tile([128, KC, DIM], BF16, name="vw_s")
            pw_s = constp.tile([128, KC, DIM], BF16, name="pw_s")
            for kc in range(KC):
                nc.sync.dma_start(qkw_s[:, kc, :], qkw[kc * 128:(kc + 1) * 128, :])
                nc.sync.dma_start(vw_s[:, kc, :], vw[kc * 128:(kc + 1) * 128, :])
                nc.sync.dma_start(pw_s[:, kc, :], pw[kc * 128:(kc + 1) * 128, :])
            eb1_s = constp.tile([128, H * NT], BF16, name="eb1_s")
            eb2_s = constp.tile([NT2, H * NT], BF16, name="eb2_s")
            nc.sync.dma_start(eb1_s[:, :], eb1[:, :])
            nc.sync.dma_start(eb2_s[:, :], eb2[:, :])
            ones_bf = constp.tile([128, 128], BF16, name="ones_bf")
            nc.gpsimd.memset(ones_bf[:, :], 1.0)
            if qkv_bias_nonzero:
                qkb_s = constp.tile([1, 2 * DIM], BF16, name="qkb_s")
                vb_s = constp.tile([1, DIM], BF16, name="vb_s")
                nc.sync.dma_start(qkb_s[:, :], qkb[:, :])
                nc.sync.dma_start(vb_s[:, :], vb[:, :])
            if proj_bias_nonzero:
                pb_s = constp.tile([1, DIM], BF16, name="pb_s")
                nc.sync.dma_start(pb_s[:, :], pb[:, :])
            if qkv_bias_nonzero or proj_bias_nonzero:
                ones_bfr = constp.tile([1, T2], BF16, name="ones_bfr")
                nc.gpsimd.memset(ones_bfr[:, :], 1.0)

            for sb in range(SB):
                # ---- load xT chunks for this batch pair ----
                xbs = []
                for kc in range(KC):
                    xbt = xbp.tile([128, T2], BF16, name="xbt", tag="xb")
                    nc.sync.dma_start(
                        xbt[:, :],
                        xt[kc * 128:(kc + 1) * 128, sb * T2:(sb + 1) * T2])
                    xbs.append(xbt)

                # ---- q,k feature-major fp32r: [1536ch -> 12 chunks, 394] ----
                qkt = []
                for c in range(2 * KC):
                    acc = ps.tile([128, T2], F32, name="acc_qk", tag="mm")
                    for kc in range(KC):
                        nc.tensor.matmul(
                            acc[:, :],
                            qkw_s[:, kc, c * 128:(c + 1) * 128],
                            xbs[kc][:, :],
                            start=(kc == 0),
                            stop=(kc == KC - 1) and not qkv_bias_nonzero,
                        )
                    if qkv_bias_nonzero:
                        nc.tensor.matmul(
                            acc[:, :],
                            qkb_s[0:1, c * 128:(c + 1) * 128],
                            ones_bfr[0:1, :],
                            start=False, stop=True,
                        )
                    t = qkp.tile([128, T2], BF16, name="qk_t", tag="qk")
                    nc.scalar.copy(t[:, :], acc[:, :])
                    qkt.append(t)

                # ---- v token-major bf16: [394 tok -> 2x(128,69), 768ch] ----
                vt = [[None, None], [None, None]]
                for b01 in range(2):
                    for tch in range(2):
                        toff = b01 * NT + tch * 128
                        tlen = 128 if tch == 0 else NT2
                        acc = ps.tile([128, DIM], F32, name="acc_v", tag="mm")
                        for half in range(2):
                            n0, n1 = half * 512, min(DIM, (half + 1) * 512)
                            for kc in range(KC):
                                nc.tensor.matmul(
                                    acc[0:tlen, n0:n1],
                                    xbs[kc][:, toff:toff + tlen],
                                    vw_s[:, kc, n0:n1],
                                    start=(kc == 0),
                                    stop=(kc == KC - 1) and not qkv_bias_nonzero,
                                )
                            if qkv_bias_nonzero:
                                nc.tensor.matmul(
                                    acc[0:tlen, n0:n1],
                                    ones_bfr[0:1, 0:tlen],
                                    vb_s[0:1, n0:n1],
                                    start=False, stop=True,
                                )
                        t = vp.tile([128, DIM], BF16, name="v_t", tag="v")
                        nc.vector.tensor_copy(t[0:tlen, :], acc[0:tlen, :])
                        vt[b01][tch] = t

                # ---- O' assembly tiles: proj rhs [768ch -> 6 chunks, 394] ----
                oprime = [opp.tile([128, T2], BF16, name="opr", tag="opr")
                          for _ in range(KC)]

                # ---- attention, software-pipelined over (h, b01) ----
                def stage_a(b01, h):
                    """S^T matmuls + exp + bias-mult -> P^T (bf16)."""
                    qc, row0 = h // 2, (h % 2) * 64
                    q_ap = qkt[qc][row0:row0 + 64, b01 * NT:(b01 + 1) * NT]
                    kt = qkt[KC + qc]
                    st = psa.tile([128, 512], F32, name="st", tag="attn")
                    nc.tensor.matmul(
                        st[:, 0:NT],
                        kt[row0:row0 + 64, b01 * NT:b01 * NT + 128],
                        q_ap, start=True, stop=True)
                    nc.tensor.matmul(
                        st[0:NT2, 256:256 + NT],
                        kt[row0:row0 + 64, b01 * NT + 128:(b01 + 1) * NT],
                        q_ap, start=True, stop=True)
                    p1 = pp.tile([128, NT], BF16, name="p1", tag="p")
                    nc.scalar.activation(
                        p1[:, :], st[:, 0:NT],
                        mybir.ActivationFunctionType.Exp)
                    p2 = pp.tile([128, NT], BF16, name="p2", tag="p")
                    nc.scalar.activation(
                        p2[0:NT2, :], st[0:NT2, 256:256 + NT],
                        mybir.ActivationFunctionType.Exp)
                    nc.vector.tensor_mul(
                        p1[:, :], p1[:, :], eb1_s[:, h * NT:(h + 1) * NT])
                    nc.vector.tensor_mul(
                        p2[0:NT2, :], p2[0:NT2, :],
                        eb2_s[:, h * NT:(h + 1) * NT])
                    return p1, p2

                def stage_b(b01, h, p1, p2):
                    """O^T + replicated sums + reciprocal + normalize."""
                    qc, row0 = h // 2, (h % 2) * 64
                    tp = (0, row0) if row0 else None
                    ot = psa.tile([128, 512], F32, name="ot", tag="attn")
                    nc.tensor.matmul(
                        ot[row0:row0 + 64, 0:NT],
                        vt[b01][0][:, h * HD:(h + 1) * HD],
                        p1[:, :], start=True, stop=False,
                        tile_position=tp)
                    nc.tensor.matmul(
                        ot[row0:row0 + 64, 0:NT],
                        vt[b01][1][0:NT2, h * HD:(h + 1) * HD],
                        p2[0:NT2, :], start=False, stop=True,
                        tile_position=tp)
                    nc.tensor.matmul(
                        ot[:, 256:256 + NT], ones_bf[:, :], p1[:, :],
                        start=True, stop=False)
                    nc.tensor.matmul(
                        ot[:, 256:256 + NT], ones_bf[0:NT2, :],
                        p2[0:NT2, :], start=False, stop=True)
                    rct = rcp.tile([128, NT], F32, name="rct", tag="rc")
                    nc.vector.reciprocal_approx_fast(
                        out=rct[:, :], in_=ot[:, 256:256 + NT])
                    nc.vector.tensor_mul(
                        oprime[qc][row0:row0 + 64, b01 * NT:(b01 + 1) * NT],
                        ot[row0:row0 + 64, 0:NT],
                        rct[row0:row0 + 64, :])

                items = [(2 * (i // 4) + (i % 2), (i // 2) % 2)
                         for i in range(2 * H)]  # (h, b01), parity-alternating
                pend = {}
                for i, (h, b01) in enumerate(items):
                    pend[i] = (b01, h) + stage_a(b01, h)
                    if i >= SKEW:
                        stage_b(*pend.pop(i - SKEW))
                for i in sorted(pend):
                    stage_b(*pend.pop(i))

                # ---- projection fp32r: out_fm [768 -> 6 chunks, 394] ----
                for c in range(KC):
                    acc = ps.tile([128, T2], F32, name="acc_p", tag="mm")
                    for kp in range(KC):
                        nc.tensor.matmul(
                            acc[:, :],
                            pw_s[:, kp, c * 128:(c + 1) * 128],
                            oprime[kp][:, :],
                            start=(kp == 0),
                            stop=(kp == KC - 1) and not proj_bias_nonzero,
                        )
                    if proj_bias_nonzero:
                        nc.tensor.matmul(
                            acc[:, :],
                            pb_s[0:1, c * 128:(c + 1) * 128],
                            ones_bfr[0:1, :],
                            start=False, stop=True,
                        )
                    obt = obp.tile([128, T2], F32, name="obt", tag="ob")
                    nc.scalar.copy(obt[:, :], acc[:, :])
                    nc.sync.dma_start(
                        out[c * 128:(c + 1) * 128, sb * T2:(sb + 1) * T2],
                        obt[:, :])

    nc.compile()
    return nc


@functools.lru_cache(maxsize=4)
def _built(qkv_bias_nonzero: bool, proj_bias_nonzero: bool):
    return build(qkv_bias_nonzero, proj_bias_nonzero)


def prepare_inputs(x, qkv_w, q_bias, v_bias, rpb_table, proj_w, proj_b, rel_index):
    """Host-side prep: shard + transpose + fold scale + gather bias table."""
    x = np.asarray(x, dtype=np.float32)
    qkv_w = np.asarray(qkv_w, dtype=np.float32)
    q_bias = np.asarray(q_bias, dtype=np.float32)
    v_bias = np.asarray(v_bias, dtype=np.float32)
    rpb_table = np.asarray(rpb_table, dtype=np.float32)
    proj_w = np.asarray(proj_w, dtype=np.float32)
    proj_b = np.asarray(proj_b, dtype=np.float32)
    rel_index = np.asarray(rel_index)

    qw = qkv_w[0:DIM] * np.float32(SCALE)   # exact: SCALE is a power of two
    qkw_h = np.ascontiguousarray(
        np.concatenate([qw, qkv_w[DIM:2 * DIM]], axis=0).T).astype(
        ml_dtypes.bfloat16)                                      # [768, 1536]
    vw_h = np.ascontiguousarray(qkv_w[2 * DIM:3 * DIM].T).astype(
        ml_dtypes.bfloat16)                                      # [768, 768]
    pw_h = np.ascontiguousarray(proj_w.T).astype(ml_dtypes.bfloat16)

    # bias[i, j, h] -> exp -> ebT[h, j, i]
    bias = rpb_table[rel_index]                                  # (197,197,12)
    ebT = np.exp(bias.astype(np.float32)).transpose(2, 1, 0)     # (12, j, i)
    eb1_h = np.ascontiguousarray(
        ebT[:, 0:128, :].transpose(1, 0, 2).reshape(128, H * NT)
    ).astype(ml_dtypes.bfloat16)
    eb2_h = np.ascontiguousarray(
        ebT[:, 128:NT, :].transpose(1, 0, 2).reshape(NT2, H * NT)
    ).astype(ml_dtypes.bfloat16)

    qkv_bias_nonzero = bool(q_bias.any() or v_bias.any())
    proj_bias_nonzero = bool(proj_b.any())

    in_maps = []
    for i in range(NCORES):
        xs = x[i * BPC:(i + 1) * BPC].reshape(BPC * NT, DIM)
        m = {
            "xt": np.ascontiguousarray(xs.T).astype(ml_dtypes.bfloat16),
            "qkw": qkw_h, "vw": vw_h, "pw": pw_h,
            "eb1": eb1_h, "eb2": eb2_h,
        }
        if qkv_bias_nonzero:
            m["qkb"] = np.ascontiguousarray(
                np.concatenate([q_bias * np.float32(SCALE),
                                np.zeros_like(q_bias)])[None, :],
                dtype=np.float32).astype(ml_dtypes.bfloat16)
            m["vb"] = np.ascontiguousarray(
                v_bias[None, :]).astype(ml_dtypes.bfloat16)
        if proj_bias_nonzero:
            m["pb"] = np.ascontiguousarray(
                proj_b[None, :], dtype=np.float32).astype(ml_dtypes.bfloat16)
        in_maps.append(m)
    return in_maps, qkv_bias_nonzero, proj_bias_nonzero


def kernel(x, qkv_w, q_bias, v_bias, rpb_table, proj_w, proj_b, rel_index):
    in_maps, qb_nz, pb_nz = prepare_inputs(
        x, qkv_w, q_bias, v_bias, rpb_table, proj_w, proj_b, rel_index)
    nc = _built(qb_nz, pb_nz)
    res = run_bass_kernel_spmd(nc, in_maps, core_ids=list(range(NCORES)))
    outs = []
    for i in range(NCORES):
        ofm = res.results[i]["out"]                  # [768, 1576]
        outs.append(ofm.T.reshape(BPC, NT, DIM))
    return np.concatenate(outs, axis=0).astype(np.float32)


# revision 6
# speedup vs baseline: 1.7444x; 1.2072x over previous
"""Trainium2 Bass kernel for windowed multi-head attention (nn_AttentionWindow).

Reference computation (B=64, N=197, DIM=768, H=12, HD=64):
    qkv  = x @ qkv_w.T + [q_bias, 0, v_bias]
    q, k, v = split(qkv);  q *= HD**-0.5
    attn = softmax(q @ k.T + rpb_table[rel_index])
    out  = (attn @ v) @ proj_w.T + proj_b

Sharding: data-parallel over batch across 8 NeuronCores (8 batches/core).

Per-core design (all matmuls on TensorE, fp32 PSUM accumulation):
  - x pre-transposed on host to xT [768, 1576] (feature-major); work
    split into 4 superbatches of one batch-pair (matmul N = 394).
  - q,k: feature-major fp32r matmuls (weights pre-scaled by 1/8 for q),
    PSUM copied to bf16 SBUF tiles (attention runs bf16).
  - v: token-major bf16 matmuls ([tokens, channels]) so the attention
    contraction has tokens on partitions; stored bf16.
  - Scores transposed: S^T[j,i] = k_h[:,j]^T q_h, bf16, N=197. Softmax
    WITHOUT max subtraction (scores are O(1): q pre-scaled), exp on
    ScalarE -> P bf16, multiplied by precomputed exp(bias) (bf16) on
    VectorE: softmax(S+B) = expS*expB / sum(expS*expB).
  - O^T[d,i] = sum_j v[j,d] P^T[j,i] (bf16); row sums replicated
    across partitions via ones-matrix matmul into the same PSUM bank;
    reciprocal_approx_fast on VectorE; normalization fused into the
    PSUM->SBUF copy assembling proj's rhs ([768, 394] per pair, f32r).
  - Attention is software-pipelined (skew over (head, batch) items,
    single-bank PSUM tiles) to keep TensorE dense and HAM-warm.
  - Projection: feature-major fp32r matmuls; host transposes back.
"""
import sys
import functools

sys.path.insert(0, "/opt/trn_rl_repo")

import numpy as np
import ml_dtypes

import concourse.bass as bass  # noqa: E402
import concourse.bacc as bacc  # noqa: E402
import concourse.mybir as mybir  # noqa: E402
from concourse.tile import TileContext  # noqa: E402
from concourse.bass_utils import run_bass_kernel_spmd  # noqa: E402

F32 = mybir.dt.float32
F32R = mybir.dt.float32r
BF16 = mybir.dt.bfloat16

NCORES = 8
B, NT, DIM = 64, 197, 768
H, HD = 12, 64
SCALE = HD ** -0.5  # 0.125, exact power of two -> folded into q weights
BPC = B // NCORES   # 8 batches per core
SB = 4              # superbatches per core (one batch-pair each)
T2 = 2 * NT         # 394
KC = DIM // 128     # 6
NT2 = NT - 128      # 69 (second token chunk)
SKEW = 1            # attention software-pipeline depth (head-pairs)


def build(qkv_bias_nonzero: bool, proj_bias_nonzero: bool):
    nc = bacc.Bacc("TRN2", target_bir_lowering=False, debug=False)

    xt = nc.dram_tensor("xt", [DIM, BPC * NT], BF16, kind="ExternalInput")
    qkw = nc.dram_tensor("qkw", [DIM, 2 * DIM], BF16, kind="ExternalInput")
    vw = nc.dram_tensor("vw", [DIM, DIM], BF16, kind="ExternalInput")
    pw = nc.dram_tensor("pw", [DIM, DIM], BF16, kind="ExternalInput")
    eb1 = nc.dram_tensor("eb1", [128, H * NT], BF16, kind="ExternalInput")
    eb2 = nc.dram_tensor("eb2", [NT2, H * NT], BF16, kind="ExternalInput")
    out = nc.dram_tensor("out", [DIM, BPC * NT], F32, kind="ExternalOutput")
    if qkv_bias_nonzero:
        qkb = nc.dram_tensor("qkb", [1, 2 * DIM], BF16, kind="ExternalInput")
        vb = nc.dram_tensor("vb", [1, DIM], BF16, kind="ExternalInput")
    if proj_bias_nonzero:
        pb = nc.dram_tensor("pb", [1, DIM], BF16, kind="ExternalInput")

    with TileContext(nc) as tc:
        with (
            tc.tile_pool(name="const", bufs=1) as constp,
            tc.tile_pool(name="xbp", bufs=9) as xbp,
            tc.tile_pool(name="qkp", bufs=18) as qkp,
            tc.tile_pool(name="vp", bufs=6) as vp,
            tc.tile_pool(name="pp", bufs=4 * (SKEW + 2)) as pp,
            tc.tile_pool(name="rcp", bufs=3) as rcp,
            tc.tile_pool(name="opp", bufs=9) as opp,
            tc.tile_pool(name="obp", bufs=3) as obp,
            tc.tile_pool(name="ps", bufs=3, space="PSUM") as ps,
            tc.tile_pool(name="psa", bufs=5, space="PSUM") as psa,
        ):
            # ---- resident constants ----
            qkw_s = constp.tile([128, KC, 2 * DIM], BF16, name="qkw_s")
            vw_s = constp.tile([128, KC, DIM], BF16, name="vw_s")
            pw_s = constp.tile([128, KC, DIM], BF16, name="pw_s")
            for kc in range(KC):
                nc.sync.dma_start(qkw_s[:, kc, :], qkw[kc * 128:(kc + 1) * 128, :])
                nc.sync.dma_start(vw_s[:, kc, :], vw[kc * 128:(kc + 1) * 128, :])
                nc.sync.dma_start(pw_s[:, kc, :], pw[kc * 128:(kc + 1) * 128, :])
            eb1_s = constp.tile([128, H * NT], BF16, name="eb1_s")
            eb2_s = constp.tile([NT2, H * NT], BF16, name="eb2_s")
            nc.sync.dma_start(eb1_s[:, :], eb1[:, :])
            nc.sync.dma_start(eb2_s[:, :], eb2[:, :])
            ones_bf = constp.tile([128, 128], BF16, name="ones_bf")
            nc.gpsimd.memset(ones_bf[:, :], 1.0)
            if qkv_bias_nonzero:
                qkb_s = constp.tile([1, 2 * DIM], BF16, name="qkb_s")
                vb_s = constp.tile([1, DIM], BF16, name="vb_s")
                nc.sync.dma_start(qkb_s[:, :], qkb[:, :])
                nc.sync.dma_start(vb_s[:, :], vb[:, :])
            if proj_bias_nonzero:
                pb_s = constp.tile([1, DIM], BF16, name="pb_s")
                nc.sync.dma_start(pb_s[:, :], pb[:, :])
            if qkv_bias_nonzero or proj_bias_nonzero:
                ones_bfr = constp.tile([1, T2], BF16, name="ones_bfr")
                nc.gpsimd.memset(ones_bfr[:, :], 1.0)

            for sb in range(SB):
                # ---- load xT chunks for this batch pair ----
                xbs = []
                for kc in range(KC):
                    xbt = xbp.tile([128, T2], BF16, name="xbt", tag="xb")
                    nc.sync.dma_start(
                        xbt[:, :],
                        xt[kc * 128:(kc + 1) * 128, sb * T2:(sb + 1) * T2])
                    xbs.append(xbt)

                # ---- q,k feature-major fp32r: [1536ch -> 12 chunks, 394] ----
                qkt = []
                for c in range(2 * KC):
                    acc = ps.tile([128, T2], F32, name="acc_qk", tag="mm")
                    for kc in range(KC):
                        nc.tensor.matmul(
                            acc[:, :],
                            qkw_s[:, kc, c * 128:(c + 1) * 128],
                            xbs[kc][:, :],
                            start=(kc == 0),
                            stop=(kc == KC - 1) and not qkv_bias_nonzero,
                        )
                    if qkv_bias_nonzero:
                        nc.tensor.matmul(
                            acc[:, :],
                            qkb_s[0:1, c * 128:(c + 1) * 128],
                            ones_bfr[0:1, :],
                            start=False, stop=True,
                        )
                    t = qkp.tile([128, T2], BF16, name="qk_t", tag="qk")
                    nc.scalar.copy(t[:, :], acc[:, :])
                    qkt.append(t)

                # ---- v token-major bf16: [394 tok -> 2x(128,69), 768ch] ----
                vt = [[None, None], [None, None]]
                for b01 in range(2):
                    for tch in range(2):
                        toff = b01 * NT + tch * 128
                        tlen = 128 if tch == 0 else NT2
                        t = vp.tile([128, DIM], BF16, name="v_t", tag="v")
                        for half in range(2):
                            n0, n1 = half * 384, (half + 1) * 384
                            acc = ps.tile([128, 384], F32, name="acc_v", tag="mm")
                            for kc in range(KC):
                                nc.tensor.matmul(
                                    acc[0:tlen, :],
                                    xbs[kc][:, toff:toff + tlen],
                                    vw_s[:, kc, n0:n1],
                                    start=(kc == 0),
                                    stop=(kc == KC - 1) and not qkv_bias_nonzero,
                                )
                            if qkv_bias_nonzero:
                                nc.tensor.matmul(
                                    acc[0:tlen, :],
                                    ones_bfr[0:1, 0:tlen],
                                    vb_s[0:1, n0:n1],
                                    start=False, stop=True,
                                )
                            nc.vector.tensor_copy(t[0:tlen, n0:n1], acc[0:tlen, :])
                        vt[b01][tch] = t

                # ---- O' assembly tiles: proj rhs [768ch -> 6 chunks, 394] ----
                oprime = [opp.tile([128, T2], BF16, name="opr", tag="opr")
                          for _ in range(KC)]

                # ---- attention, software-pipelined over (head-pair, b01) ----
                def stage_a(b01, hp):
                    """Score matmuls for heads 2hp,2hp+1 (row-packed) + exp
                    + bias-mult -> P^T tiles (bf16)."""
                    h0, h1 = 2 * hp, 2 * hp + 1
                    kt = qkt[KC + hp]
                    q0 = qkt[hp][0:64, b01 * NT:(b01 + 1) * NT]
                    q1 = qkt[hp][64:128, b01 * NT:(b01 + 1) * NT]
                    st0 = psa.tile([128, 512], F32, name="st0", tag="attn")
                    st1 = psa.tile([128, 512], F32, name="st1", tag="attn")
                    # j1/j2 chunks, heads interleaved so lhsT row-halves
                    # alternate (array row-packing)
                    nc.tensor.matmul(st0[:, 0:NT],
                                     kt[0:64, b01 * NT:b01 * NT + 128],
                                     q0, start=True, stop=True)
                    nc.tensor.matmul(st1[:, 0:NT],
                                     kt[64:128, b01 * NT:b01 * NT + 128],
                                     q1, start=True, stop=True)
                    nc.tensor.matmul(st0[0:NT2, 256:256 + NT],
                                     kt[0:64, b01 * NT + 128:(b01 + 1) * NT],
                                     q0, start=True, stop=True)
                    nc.tensor.matmul(st1[0:NT2, 256:256 + NT],
                                     kt[64:128, b01 * NT + 128:(b01 + 1) * NT],
                                     q1, start=True, stop=True)
                    ps_ = []
                    for h, st in ((h0, st0), (h1, st1)):
                        p1 = pp.tile([128, NT], BF16, name="p1", tag="p")
                        nc.scalar.activation(
                            p1[:, :], st[:, 0:NT],
                            mybir.ActivationFunctionType.Exp)
                        p2 = pp.tile([128, NT], BF16, name="p2", tag="p")
                        nc.scalar.activation(
                            p2[0:NT2, :], st[0:NT2, 256:256 + NT],
                            mybir.ActivationFunctionType.Exp)
                        nc.vector.tensor_mul(
                            p1[:, :], p1[:, :], eb1_s[:, h * NT:(h + 1) * NT])
                        nc.vector.tensor_mul(
                            p2[0:NT2, :], p2[0:NT2, :],
                            eb2_s[:, h * NT:(h + 1) * NT])
                        ps_ += [p1, p2]
                    return ps_

                def stage_b(b01, hp, p10, p20, p11, p21):
                    """O^T (col-packed head pair) + sums + recip + norm."""
                    h0, h1 = 2 * hp, 2 * hp + 1
                    ot = psa.tile([128, 512], F32, name="ot", tag="attn")
                    su = psa.tile([128, 512], F32, name="su", tag="attn")
                    nc.tensor.matmul(
                        ot[0:64, 0:NT],
                        vt[b01][0][:, h0 * HD:(h0 + 1) * HD],
                        p10[:, :], start=True, stop=False)
                    nc.tensor.matmul(
                        ot[64:128, 0:NT],
                        vt[b01][0][:, h1 * HD:(h1 + 1) * HD],
                        p11[:, :], start=True, stop=False,
                        tile_position=(0, 64))
                    nc.tensor.matmul(
                        ot[0:64, 0:NT],
                        vt[b01][1][0:NT2, h0 * HD:(h0 + 1) * HD],
                        p20[0:NT2, :], start=False, stop=True)
                    nc.tensor.matmul(
                        ot[64:128, 0:NT],
                        vt[b01][1][0:NT2, h1 * HD:(h1 + 1) * HD],
                        p21[0:NT2, :], start=False, stop=True,
                        tile_position=(0, 64))
                    nc.tensor.matmul(su[:, 0:NT], ones_bf[:, :],
                                     p10[:, :], start=True, stop=False)
                    nc.tensor.matmul(su[:, 0:NT], ones_bf[0:NT2, :],
                                     p20[0:NT2, :], start=False, stop=True)
                    nc.tensor.matmul(su[:, 256:256 + NT], ones_bf[:, :],
                                     p11[:, :], start=True, stop=False)
                    nc.tensor.matmul(su[:, 256:256 + NT], ones_bf[0:NT2, :],
                                     p21[0:NT2, :], start=False, stop=True)
                    rc0 = rcp.tile([128, NT], F32, name="rc0", tag="rc")
                    nc.vector.reciprocal_approx_fast(
                        out=rc0[:, :], in_=su[:, 0:NT])
                    rc1 = rcp.tile([128, NT], F32, name="rc1", tag="rc")
                    nc.vector.reciprocal_approx_fast(
                        out=rc1[:, :], in_=su[:, 256:256 + NT])
                    nc.vector.tensor_mul(
                        oprime[hp][0:64, b01 * NT:(b01 + 1) * NT],
                        ot[0:64, 0:NT], rc0[0:64, :])
                    nc.vector.tensor_mul(
                        oprime[hp][64:128, b01 * NT:(b01 + 1) * NT],
                        ot[64:128, 0:NT], rc1[64:128, :])

                items = [(i // 2, i % 2) for i in range(H)]  # (hp, b01)
                pend = {}
                for i, (hp, b01) in enumerate(items):
                    pend[i] = (b01, hp) + tuple(stage_a(b01, hp))
                    if i >= SKEW:
                        stage_b(*pend.pop(i - SKEW))
                for i in sorted(pend):
                    stage_b(*pend.pop(i))

                # ---- projection fp32r: out_fm [768 -> 6 chunks, 394] ----
                for c in range(KC):
                    acc = ps.tile([128, T2], F32, name="acc_p", tag="mm")
                    for kp in range(KC):
                        nc.tensor.matmul(
                            acc[:, :],
                            pw_s[:, kp, c * 128:(c + 1) * 128],
                            oprime[kp][:, :],
                            start=(kp == 0),
                            stop=(kp == KC - 1) and not proj_bias_nonzero,
                        )
                    if proj_bias_nonzero:
                        nc.tensor.matmul(
                            acc[:, :],
                            pb_s[0:1, c * 128:(c + 1) * 128],
                            ones_bfr[0:1, :],
                            start=False, stop=True,
                        )
                    obt = obp.tile([128, T2], F32, name="obt", tag="ob")
                    nc.scalar.copy(obt[:, :], acc[:, :])
                    nc.sync.dma_start(
                        out[c * 128:(c + 1) * 128, sb * T2:(sb + 1) * T2],
                        obt[:, :])

    nc.compile()
    return nc


@functools.lru_cache(maxsize=4)
def _built(qkv_bias_nonzero: bool, proj_bias_nonzero: bool):
    return build(qkv_bias_nonzero, proj_bias_nonzero)


def prepare_inputs(x, qkv_w, q_bias, v_bias, rpb_table, proj_w, proj_b, rel_index):
    """Host-side prep: shard + transpose + fold scale + gather bias table."""
    x = np.asarray(x, dtype=np.float32)
    qkv_w = np.asarray(qkv_w, dtype=np.float32)
    q_bias = np.asarray(q_bias, dtype=np.float32)
    v_bias = np.asarray(v_bias, dtype=np.float32)
    rpb_table = np.asarray(rpb_table, dtype=np.float32)
    proj_w = np.asarray(proj_w, dtype=np.float32)
    proj_b = np.asarray(proj_b, dtype=np.float32)
    rel_index = np.asarray(rel_index)

    qw = qkv_w[0:DIM] * np.float32(SCALE)   # exact: SCALE is a power of two
    qkw_h = np.ascontiguousarray(
        np.concatenate([qw, qkv_w[DIM:2 * DIM]], axis=0).T).astype(
        ml_dtypes.bfloat16)                                      # [768, 1536]
    vw_h = np.ascontiguousarray(qkv_w[2 * DIM:3 * DIM].T).astype(
        ml_dtypes.bfloat16)                                      # [768, 768]
    pw_h = np.ascontiguousarray(proj_w.T).astype(ml_dtypes.bfloat16)

    # bias[i, j, h] -> exp -> ebT[h, j, i]
    bias = rpb_table[rel_index]                                  # (197,197,12)
    ebT = np.exp(bias.astype(np.float32)).transpose(2, 1, 0)     # (12, j, i)
    eb1_h = np.ascontiguousarray(
        ebT[:, 0:128, :].transpose(1, 0, 2).reshape(128, H * NT)
    ).astype(ml_dtypes.bfloat16)
    eb2_h = np.ascontiguousarray(
        ebT[:, 128:NT, :].transpose(1, 0, 2).reshape(NT2, H * NT)
    ).astype(ml_dtypes.bfloat16)

    qkv_bias_nonzero = bool(q_bias.any() or v_bias.any())
    proj_bias_nonzero = bool(proj_b.any())

    in_maps = []
    for i in range(NCORES):
        xs = x[i * BPC:(i + 1) * BPC].reshape(BPC * NT, DIM)
        m = {
            "xt": np.ascontiguousarray(xs.T).astype(ml_dtypes.bfloat16),
            "qkw": qkw_h, "vw": vw_h, "pw": pw_h,
            "eb1": eb1_h, "eb2": eb2_h,
        }
        if qkv_bias_nonzero:
            m["qkb"] = np.ascontiguousarray(
                np.concatenate([q_bias * np.float32(SCALE),
                                np.zeros_like(q_bias)])[None, :],
                dtype=np.float32).astype(ml_dtypes.bfloat16)
            m["vb"] = np.ascontiguousarray(
                v_bias[None, :]).astype(ml_dtypes.bfloat16)
        if proj_bias_nonzero:
            m["pb"] = np.ascontiguousarray(
                proj_b[None, :], dtype=np.float32).astype(ml_dtypes.bfloat16)
        in_maps.append(m)
    return in_maps, qkv_bias_nonzero, proj_bias_nonzero


def kernel(x, qkv_w, q_bias, v_bias, rpb_table, proj_w, proj_b, rel_index):
    in_maps, qb_nz, pb_nz = prepare_inputs(
        x, qkv_w, q_bias, v_bias, rpb_table, proj_w, proj_b, rel_index)
    nc = _built(qb_nz, pb_nz)
    res = run_bass_kernel_spmd(nc, in_maps, core_ids=list(range(NCORES)))
    outs = []
    for i in range(NCORES):
        ofm = res.results[i]["out"]                  # [768, 1576]
        outs.append(ofm.T.reshape(BPC, NT, DIM))
    return np.concatenate(outs, axis=0).astype(np.float32)


# revision 9
# speedup vs baseline: 1.9108x; 1.0954x over previous
"""Trainium2 Bass kernel for windowed multi-head attention (nn_AttentionWindow).

Reference computation (B=64, N=197, DIM=768, H=12, HD=64):
    qkv  = x @ qkv_w.T + [q_bias, 0, v_bias]
    q, k, v = split(qkv);  q *= HD**-0.5
    attn = softmax(q @ k.T + rpb_table[rel_index])
    out  = (attn @ v) @ proj_w.T + proj_b

Sharding: data-parallel over batch across 8 NeuronCores (8 batches/core).

Per-core design (bf16 matmuls on TensorE, fp32 PSUM accumulation):
  - x pre-transposed on host to xT [768, 1576] bf16 (feature-major),
    resident in SBUF. q,k computed feature-major into one resident
    [128, 12, 1576] tile, tiled over 512-token slices (batch-agnostic).
  - v: token-major per batch ([tokens, channels], 128+69 row chunks) so
    the attention contraction has tokens on partitions; stored bf16.
  - Scores transposed: S^T[j,i] = k_h[:,j]^T q_h, per (head-pair,
    batch) item. The two heads of a pair use opposite 64-row halves of
    the PE array (row-packing -> concurrent matmuls). Softmax WITHOUT
    max subtraction (scores are O(1): q pre-scaled by 1/8): exp on
    ScalarE (both heads' score chunks merged into one strided
    activation) -> P^T bf16, then one VectorE multiply against the
    precomputed exp(bias) table: softmax(S+B) = expS*expB / sums.
  - O^T[d,i] = sum_j v[j,d] P^T[j,i]: head pair col-packed via
    tile_position (0,0)/(0,64); softmax denominators via ones-matrix
    matmuls (replicated across partitions in PSUM);
    reciprocal_approx_fast on VectorE; normalization fused into the
    PSUM->SBUF copy assembling proj's rhs (resident [128, 6, 1576]).
  - Attention software-pipelined (skew 2) to keep TensorE dense.
  - Projection: feature-major over 512-token slices; host transposes.
"""
import sys
import functools

sys.path.insert(0, "/opt/trn_rl_repo")

import numpy as np
import ml_dtypes

import concourse.bass as bass  # noqa: E402
import concourse.bacc as bacc  # noqa: E402
import concourse.mybir as mybir  # noqa: E402
from concourse.tile import TileContext  # noqa: E402
from concourse.bass_utils import run_bass_kernel_spmd  # noqa: E402

F32 = mybir.dt.float32
BF16 = mybir.dt.bfloat16

NCORES = 8
B, NT, DIM = 64, 197, 768
H, HD = 12, 64
SCALE = HD ** -0.5  # 0.125, exact power of two -> folded into q weights
BPC = B // NCORES   # 8 batches per core
TOK = BPC * NT      # 1576 tokens per core
KC = DIM // 128     # 6
NT2 = NT - 128      # 69 (second token chunk)
SKEW = 2            # attention software-pipeline depth (items)
# 512-token slices for the token-parallel (qk, proj) matmuls
SLICES = [(s * 512, min(TOK, (s + 1) * 512)) for s in range((TOK + 511) // 512)]


def build(qkv_bias_nonzero: bool, proj_bias_nonzero: bool):
    nc = bacc.Bacc("TRN2", target_bir_lowering=False, debug=False)

    xt = nc.dram_tensor("xt", [DIM, TOK], BF16, kind="ExternalInput")
    qkw = nc.dram_tensor("qkw", [DIM, 2 * DIM], BF16, kind="ExternalInput")
    vw = nc.dram_tensor("vw", [DIM, DIM], BF16, kind="ExternalInput")
    pw = nc.dram_tensor("pw", [DIM, DIM], BF16, kind="ExternalInput")
    eb1 = nc.dram_tensor("eb1", [128, H * NT], BF16, kind="ExternalInput")
    eb2 = nc.dram_tensor("eb2", [NT2, H * NT], BF16, kind="ExternalInput")
    out = nc.dram_tensor("out", [DIM, TOK], F32, kind="ExternalOutput")
    if qkv_bias_nonzero:
        qkb = nc.dram_tensor("qkb", [1, 2 * DIM], BF16, kind="ExternalInput")
        vb = nc.dram_tensor("vb", [1, DIM], BF16, kind="ExternalInput")
    if proj_bias_nonzero:
        pb = nc.dram_tensor("pb", [1, DIM], BF16, kind="ExternalInput")

    with TileContext(nc) as tc:
        with (
            tc.tile_pool(name="const", bufs=1) as constp,
            tc.tile_pool(name="vp", bufs=16) as vp,
            tc.tile_pool(name="pp", bufs=2 * (SKEW + 2)) as pp,
            tc.tile_pool(name="rcp", bufs=4) as rcp,
            tc.tile_pool(name="obp", bufs=4) as obp,
            tc.tile_pool(name="ps", bufs=4, space="PSUM") as ps,
            tc.tile_pool(name="sta", bufs=2, space="PSUM") as sta,
        ):
            # ---- resident constants & activations ----
            xb_s = constp.tile([128, KC, TOK], BF16, name="xb_s")
            qkw_s = constp.tile([128, KC, 2 * DIM], BF16, name="qkw_s")
            vw_s = constp.tile([128, KC, DIM], BF16, name="vw_s")
            pw_s = constp.tile([128, KC, DIM], BF16, name="pw_s")
            for kc in range(KC):
                nc.sync.dma_start(xb_s[:, kc, :], xt[kc * 128:(kc + 1) * 128, :])
                nc.sync.dma_start(qkw_s[:, kc, :], qkw[kc * 128:(kc + 1) * 128, :])
                nc.sync.dma_start(vw_s[:, kc, :], vw[kc * 128:(kc + 1) * 128, :])
                nc.sync.dma_start(pw_s[:, kc, :], pw[kc * 128:(kc + 1) * 128, :])
            eb1_s = constp.tile([128, H * NT], BF16, name="eb1_s")
            eb2_s = constp.tile([NT2, H * NT], BF16, name="eb2_s")
            nc.sync.dma_start(eb1_s[:, :], eb1[:, :])
            nc.sync.dma_start(eb2_s[:, :], eb2[:, :])
            ones_bf = constp.tile([128, 128], BF16, name="ones_bf")
            nc.gpsimd.memset(ones_bf[:, :], 1.0)
            # big resident activations: q,k and proj-rhs (bf16)
            qk_s = constp.tile([128, 2 * KC, TOK], BF16, name="qk_s")
            op_s = constp.tile([128, KC, TOK], BF16, name="op_s")
            if qkv_bias_nonzero:
                qkb_s = constp.tile([1, 2 * DIM], BF16, name="qkb_s")
                vb_s = constp.tile([1, DIM], BF16, name="vb_s")
                nc.sync.dma_start(qkb_s[:, :], qkb[:, :])
                nc.sync.dma_start(vb_s[:, :], vb[:, :])
            if proj_bias_nonzero:
                pb_s = constp.tile([1, DIM], BF16, name="pb_s")
                nc.sync.dma_start(pb_s[:, :], pb[:, :])
            if qkv_bias_nonzero or proj_bias_nonzero:
                ones_bfr = constp.tile([1, 512], BF16, name="ones_bfr")
                nc.gpsimd.memset(ones_bfr[:, :], 1.0)

            # ---- q,k feature-major: 12 channel-chunks x token slices ----
            for c in range(2 * KC):
                for t0, t1 in SLICES:
                    acc = ps.tile([128, 512], F32, name="acc_qk", tag="mm")
                    w = t1 - t0
                    for kc in range(KC):
                        nc.tensor.matmul(
                            acc[:, 0:w],
                            qkw_s[:, kc, c * 128:(c + 1) * 128],
                            xb_s[:, kc, t0:t1],
                            start=(kc == 0),
                            stop=(kc == KC - 1) and not qkv_bias_nonzero,
                        )
                    if qkv_bias_nonzero:
                        nc.tensor.matmul(
                            acc[:, 0:w],
                            qkb_s[0:1, c * 128:(c + 1) * 128],
                            ones_bfr[0:1, 0:w],
                            start=False, stop=True,
                        )
                    nc.scalar.copy(qk_s[:, c, t0:t1], acc[:, 0:w])

            # ---- v token-major per batch: [(128|69) tok, 768 ch] ----
            vt = [[None, None] for _ in range(BPC)]
            for b in range(BPC):
                for tch in range(2):
                    toff = b * NT + tch * 128
                    tlen = 128 if tch == 0 else NT2
                    t = vp.tile([128, DIM], BF16, name="v_t", tag="v")
                    for half in range(2):
                        n0, n1 = half * 384, (half + 1) * 384
                        acc = ps.tile([128, 384], F32, name="acc_v", tag="mm")
                        for kc in range(KC):
                            nc.tensor.matmul(
                                acc[0:tlen, :],
                                xb_s[:, kc, toff:toff + tlen],
                                vw_s[:, kc, n0:n1],
                                start=(kc == 0),
                                stop=(kc == KC - 1) and not qkv_bias_nonzero,
                            )
                        if qkv_bias_nonzero:
                            nc.tensor.matmul(
                                acc[0:tlen, :],
                                ones_bfr[0:1, 0:tlen],
                                vb_s[0:1, n0:n1],
                                start=False, stop=True,
                            )
                        nc.vector.tensor_copy(t[0:tlen, n0:n1], acc[0:tlen, :])
                    vt[b][tch] = t

            # ---- attention, software-pipelined over (head-pair, batch) ----
            def stage_a(b, hp):
                """Scores for heads 2hp,2hp+1 (array row-packed), merged
                exp + merged bias-mult -> P^T pair tiles (bf16)."""
                h0 = 2 * hp
                st = sta.tile([128, 1024], F32, name="st", tag="sta")
                q0 = qk_s[0:64, hp, b * NT:(b + 1) * NT]
                q1 = qk_s[64:128, hp, b * NT:(b + 1) * NT]
                # j1 chunks (K rows 0:64 then 64:128 -> packed), then j2
                nc.tensor.matmul(st[:, 0:NT],
                                 qk_s[0:64, KC + hp, b * NT:b * NT + 128],
                                 q0, start=True, stop=True)
                nc.tensor.matmul(st[:, 512:512 + NT],
                                 qk_s[64:128, KC + hp, b * NT:b * NT + 128],
                                 q1, start=True, stop=True)
                nc.tensor.matmul(st[0:NT2, 256:256 + NT],
                                 qk_s[0:64, KC + hp, b * NT + 128:(b + 1) * NT],
                                 q0, start=True, stop=True)
                nc.tensor.matmul(st[0:NT2, 768:768 + NT],
                                 qk_s[64:128, KC + hp, b * NT + 128:(b + 1) * NT],
                                 q1, start=True, stop=True)
                # merged exp over both heads (strided free AP), bf16 out
                pj1 = pp.tile([128, 2, NT], BF16, name="pj1", tag="p")
                nc.scalar.activation(
                    pj1[:, :, :], st[:, 0:1024].rearrange("p (h x) -> p h x", h=2)[:, :, 0:NT],
                    mybir.ActivationFunctionType.Exp)
                pj2 = pp.tile([128, 2, NT], BF16, name="pj2", tag="p")
                nc.scalar.activation(
                    pj2[0:NT2, :, :],
                    st[0:NT2, 0:1024].rearrange("p (h x) -> p h x", h=2)[:, :, 256:256 + NT],
                    mybir.ActivationFunctionType.Exp)
                # merged bias multiply (heads adjacent in eb tables)
                nc.vector.tensor_mul(
                    pj1[:, :, :].rearrange("p h x -> p (h x)"),
                    pj1[:, :, :].rearrange("p h x -> p (h x)"),
                    eb1_s[:, h0 * NT:(h0 + 2) * NT])
                nc.vector.tensor_mul(
                    pj2[0:NT2, :, :].rearrange("p h x -> p (h x)"),
                    pj2[0:NT2, :, :].rearrange("p h x -> p (h x)"),
                    eb2_s[:, h0 * NT:(h0 + 2) * NT])
                return pj1, pj2

            def stage_b(b, hp, pj1, pj2):
                """O^T (head pair col-packed) + sums + recip + norm."""
                h0, h1 = 2 * hp, 2 * hp + 1
                ot = ps.tile([128, 512], F32, name="ot", tag="mm")
                su = ps.tile([128, 512], F32, name="su", tag="mm")
                nc.tensor.matmul(
                    ot[0:64, 0:NT],
                    vt[b][0][:, h0 * HD:(h0 + 1) * HD],
                    pj1[:, 0, :], start=True, stop=False)
                nc.tensor.matmul(
                    ot[64:128, 0:NT],
                    vt[b][0][:, h1 * HD:(h1 + 1) * HD],
                    pj1[:, 1, :], start=True, stop=False,
                    tile_position=(0, 64))
                nc.tensor.matmul(
                    ot[0:64, 0:NT],
                    vt[b][1][0:NT2, h0 * HD:(h0 + 1) * HD],
                    pj2[0:NT2, 0, :], start=False, stop=True)
                nc.tensor.matmul(
                    ot[64:128, 0:NT],
                    vt[b][1][0:NT2, h1 * HD:(h1 + 1) * HD],
                    pj2[0:NT2, 1, :], start=False, stop=True,
                    tile_position=(0, 64))
                nc.tensor.matmul(su[:, 0:NT], ones_bf[:, :],
                                 pj1[:, 0, :], start=True, stop=False)
                nc.tensor.matmul(su[:, 0:NT], ones_bf[0:NT2, :],
                                 pj2[0:NT2, 0, :], start=False, stop=True)
                nc.tensor.matmul(su[:, 256:256 + NT], ones_bf[:, :],
                                 pj1[:, 1, :], start=True, stop=False)
                nc.tensor.matmul(su[:, 256:256 + NT], ones_bf[0:NT2, :],
                                 pj2[0:NT2, 1, :], start=False, stop=True)
                rc0 = rcp.tile([128, NT], F32, name="rc0", tag="rc")
                nc.vector.reciprocal_approx_fast(out=rc0[:, :], in_=su[:, 0:NT])
                rc1 = rcp.tile([128, NT], F32, name="rc1", tag="rc")
                nc.vector.reciprocal_approx_fast(
                    out=rc1[:, :], in_=su[:, 256:256 + NT])
                nc.vector.tensor_mul(
                    op_s[0:64, hp, b * NT:(b + 1) * NT],
                    ot[0:64, 0:NT], rc0[0:64, :])
                nc.vector.tensor_mul(
                    op_s[64:128, hp, b * NT:(b + 1) * NT],
                    ot[64:128, 0:NT], rc1[64:128, :])

            items = [(hp, b) for b in range(BPC) for hp in range(KC)]
            pend = {}
            for i, (hp, b) in enumerate(items):
                pend[i] = (b, hp) + tuple(stage_a(b, hp))
                if i >= SKEW:
                    stage_b(*pend.pop(i - SKEW))
            for i in sorted(pend):
                stage_b(*pend.pop(i))

            # ---- projection: out_fm [768 -> 6 chunks, token slices] ----
            for c in range(KC):
                for t0, t1 in SLICES:
                    acc = ps.tile([128, 512], F32, name="acc_p", tag="mm")
                    w = t1 - t0
                    for kp in range(KC):
                        nc.tensor.matmul(
                            acc[:, 0:w],
                            pw_s[:, kp, c * 128:(c + 1) * 128],
                            op_s[:, kp, t0:t1],
                            start=(kp == 0),
                            stop=(kp == KC - 1) and not proj_bias_nonzero,
                        )
                    if proj_bias_nonzero:
                        nc.tensor.matmul(
                            acc[:, 0:w],
                            pb_s[0:1, c * 128:(c + 1) * 128],
                            ones_bfr[0:1, 0:w],
                            start=False, stop=True,
                        )
                    obt = obp.tile([128, 512], F32, name="obt", tag="ob")
                    nc.scalar.copy(obt[:, 0:w], acc[:, 0:w])
                    nc.sync.dma_start(out[c * 128:(c + 1) * 128, t0:t1],
                                      obt[:, 0:w])

    nc.compile()
    return nc


@functools.lru_cache(maxsize=4)
def _built(qkv_bias_nonzero: bool, proj_bias_nonzero: bool):
    return build(qkv_bias_nonzero, proj_bias_nonzero)


def prepare_inputs(x, qkv_w, q_bias, v_bias, rpb_table, proj_w, proj_b, rel_index):
    """Host-side prep: shard + transpose + fold scale + gather bias table."""
    x = np.asarray(x, dtype=np.float32)
    qkv_w = np.asarray(qkv_w, dtype=np.float32)
    q_bias = np.asarray(q_bias, dtype=np.float32)
    v_bias = np.asarray(v_bias, dtype=np.float32)
    rpb_table = np.asarray(rpb_table, dtype=np.float32)
    proj_w = np.asarray(proj_w, dtype=np.float32)
    proj_b = np.asarray(proj_b, dtype=np.float32)
    rel_index = np.asarray(rel_index)

    qw = qkv_w[0:DIM] * np.float32(SCALE)   # exact: SCALE is a power of two
    qkw_h = np.ascontiguousarray(
        np.concatenate([qw, qkv_w[DIM:2 * DIM]], axis=0).T).astype(
        ml_dtypes.bfloat16)                                      # [768, 1536]
    vw_h = np.ascontiguousarray(qkv_w[2 * DIM:3 * DIM].T).astype(
        ml_dtypes.bfloat16)                                      # [768, 768]
    pw_h = np.ascontiguousarray(proj_w.T).astype(ml_dtypes.bfloat16)

    # bias[i, j, h] -> exp -> ebT[h, j, i]
    bias = rpb_table[rel_index]                                  # (197,197,12)
    ebT = np.exp(bias.astype(np.float32)).transpose(2, 1, 0)     # (12, j, i)
    eb1_h = np.ascontiguousarray(
        ebT[:, 0:128, :].transpose(1, 0, 2).reshape(128, H * NT)
    ).astype(ml_dtypes.bfloat16)
    eb2_h = np.ascontiguousarray(
        ebT[:, 128:NT, :].transpose(1, 0, 2).reshape(NT2, H * NT)
    ).astype(ml_dtypes.bfloat16)

    qkv_bias_nonzero = bool(q_bias.any() or v_bias.any())
    proj_bias_nonzero = bool(proj_b.any())

    in_maps = []
    for i in range(NCORES):
        xs = x[i * BPC:(i + 1) * BPC].reshape(TOK, DIM)
        m = {
            "xt": np.ascontiguousarray(xs.T).astype(ml_dtypes.bfloat16),
            "qkw": qkw_h, "vw": vw_h, "pw": pw_h,
            "eb1": eb1_h, "eb2": eb2_h,
        }
        if qkv_bias_nonzero:
            m["qkb"] = np.ascontiguousarray(
                np.concatenate([q_bias * np.float32(SCALE),
                                np.zeros_like(q_bias)])[None, :],
                dtype=np.float32).astype(ml_dtypes.bfloat16)
            m["vb"] = np.ascontiguousarray(
                v_bias[None, :]).astype(ml_dtypes.bfloat16)
        if proj_bias_nonzero:
            m["pb"] = np.ascontiguousarray(
                proj_b[None, :], dtype=np.float32).astype(ml_dtypes.bfloat16)
        in_maps.append(m)
    return in_maps, qkv_bias_nonzero, proj_bias_nonzero


def kernel(x, qkv_w, q_bias, v_bias, rpb_table, proj_w, proj_b, rel_index):
    in_maps, qb_nz, pb_nz = prepare_inputs(
        x, qkv_w, q_bias, v_bias, rpb_table, proj_w, proj_b, rel_index)
    nc = _built(qb_nz, pb_nz)
    res = run_bass_kernel_spmd(nc, in_maps, core_ids=list(range(NCORES)))
    outs = []
    for i in range(NCORES):
        ofm = res.results[i]["out"]                  # [768, 1576]
        outs.append(ofm.T.reshape(BPC, NT, DIM))
    return np.concatenate(outs, axis=0).astype(np.float32)
